# revision 6
# baseline (speedup 1.0000x reference)
"""nn_BlockPositioning: out[b*8+h, i, j] = ev_h[i//4, j//4] + c_h[i%4, j%4]

with ev_h[a, b] = eb_h[a-b] if a>b else ebf_h[b-a]  (Toeplitz in a-b); the
batch axis is a pure tile of the per-head bias.  Sharding: one head per core
(8 heads, 8 cores); the 4 identical batch copies are materialized host-side
at gather time.

Per-core device program (pure data movement + one fp32 add per unique value):
  Grev[s] = g_h[E-1-s]                       host layout prep (reverse+concat),
                                             replicated to 128 partitions
  S[p, 4s+jr] = Grev[s] + c_h[p%4, jr]       4x tensor_scalar_add on DVE
                                             (c replicated so row p%4 rides
                                             partition p's lane)
  out[128t+4q+r, j] = S[4q+r, (2044-4q-128t)+j]
                                             32 window DMAs (one per q);
                                             every output row is one
                                             contiguous 8 KiB read

The output window start depends only on i//4 = 32t+q, so for a fixed
partition-group q all 64 rows it serves are plain strided windows - each DMA
is [partition-pitch, -128 free step, contiguous 8 KiB], a shape verified on
hardware.  Bulk traffic is write-only: ~16 MiB/core at the HBM roofline.
"""

import numpy as np

_H = 8
_B = 4
_E = 512
_SEQ = 4 * _E              # 2048
_GLEN = 2 * _E - 1         # 1023
_GI_LEN = 4 * _GLEN        # 4092
_NT = _SEQ // 128          # 16
_NQ = 32                   # partition groups of 4

_CACHE = {}


def _build_nc():
    import concourse.bass as bass
    import concourse.mybir as mybir

    F32 = mybir.dt.float32
    nc = bass.Bass()
    grev_in = nc.dram_tensor("grev", [128, _GLEN], F32, kind="ExternalInput")
    cmat_in = nc.dram_tensor("cmat", [128, 4], F32, kind="ExternalInput")
    out = nc.dram_tensor("out", [_SEQ, _SEQ], F32, kind="ExternalOutput")

    n_dma = 0

    with (
        nc.sbuf_tensor([128, _GLEN], F32) as grev_sb,
        nc.sbuf_tensor([128, 4], F32) as c_sb,
        nc.sbuf_tensor([128, _GI_LEN], F32) as s2,
        nc.semaphore("dma_sem") as dma_sem,
        nc.semaphore("v_sem") as v_sem,
        nc.Block() as block,
    ):
        # S[p, 4s+jr] = Grev[s] + c[p%4, jr]   (strided dest view)
        s3 = s2[:, :].rearrange("p (s j) -> p s j", j=4)

        @block.vector
        def _(vector):
            vector.wait_ge(dma_sem, 32)  # grev + cmat resident
            for jr in range(4):
                vector.tensor_scalar_add(
                    out=s3[:, :, jr],
                    in0=grev_sb[:, :],
                    scalar1=c_sb[:, jr : jr + 1],
                ).then_inc(v_sem, 1)

        @block.sync
        def _(sync):
            nonlocal n_dma
            sync.dma_start(out=grev_sb[:, :], in_=grev_in[:, :]).then_inc(dma_sem, 16)
            sync.dma_start(out=c_sb[:, :], in_=cmat_in[:, :]).then_inc(dma_sem, 16)
            n_dma = 2
            sync.wait_ge(v_sem, 4)  # S computed
            # out[128t + 4q + r, j] = S[4q+r, (2044 - 4q - 128t) + j]
            for q in range(_NQ):
                sb = s2[4 * q : 4 * q + 4, :]
                src = bass.AP(
                    sb.tensor,
                    sb.offset + (_GI_LEN - _SEQ - 4 * q),
                    [[_GI_LEN, 4], [-128, _NT], [1, _SEQ]],
                )
                dst = bass.AP(
                    out[:, :].tensor,
                    4 * q * _SEQ,
                    [[_SEQ, 4], [128 * _SEQ, _NT], [1, _SEQ]],
                )
                with nc.allow_non_contiguous_dma(reason="toeplitz windows"):
                    sync.dma_start(out=dst, in_=src).then_inc(dma_sem, 16)
                n_dma += 1
            sync.wait_ge(dma_sem, 16 * n_dma)

    return nc


def _in_maps(channel_blocks, event_blocks, event_blocks_future):
    maps = []
    for h in range(_H):
        eb = np.ascontiguousarray(event_blocks[:, 0, h], dtype=np.float32)
        ebf = np.ascontiguousarray(event_blocks_future[:, 0, h], dtype=np.float32)
        grev = np.concatenate([eb[_E - 1 : 0 : -1], ebf])
        c = np.ascontiguousarray(channel_blocks[:, :, 0, h], dtype=np.float32)  # (4,4)
        maps.append(
            {
                "grev": np.ascontiguousarray(
                    np.broadcast_to(grev, (128, _GLEN)), dtype=np.float32
                ),
                "cmat": np.ascontiguousarray(
                    np.tile(c, (32, 1)), dtype=np.float32
                ),  # row p -> c[p%4, :]
            }
        )
    return maps


def run_spmd(channel_blocks, event_blocks, event_blocks_future, **spmd_kwargs):
    """Run the per-head kernels on cores 0-7; returns (BassKernelResults, heads).

    heads: float32 (8, 2048, 2048), one bias matrix per head."""
    from concourse.bass_utils import run_bass_kernel_spmd

    if "nc" not in _CACHE:
        _CACHE["nc"] = _build_nc()
    res = run_bass_kernel_spmd(
        _CACHE["nc"],
        _in_maps(channel_blocks, event_blocks, event_blocks_future),
        list(range(_H)),
        **spmd_kwargs,
    )
    heads = np.stack([np.asarray(res.results[h]["out"]) for h in range(_H)])
    return res, heads


def kernel(q, channel_blocks, event_blocks, event_blocks_future):
    q = np.asarray(q)
    channel_blocks = np.asarray(channel_blocks, dtype=np.float32)
    event_blocks = np.asarray(event_blocks, dtype=np.float32)
    event_blocks_future = np.asarray(event_blocks_future, dtype=np.float32)

    _, heads = run_spmd(channel_blocks, event_blocks, event_blocks_future)
    batch = q.shape[0] // _H
    return np.tile(heads, (batch, 1, 1))


# revision 7
# speedup vs baseline: 4.3977x; 4.3977x over previous
"""nn_BlockPositioning: out[b*8+h, i, j] = ev_h[i//4, j//4] + c_h[i%4, j%4]

with ev_h[a, b] = eb_h[a-b] if a>b else ebf_h[b-a]  (Toeplitz in a-b); the
batch axis is a pure tile of the per-head bias.  Sharding: one head per core
(8 heads, 8 cores); the 4 identical batch copies are materialized host-side
at gather time.

Per-core device program (pure data movement + one fp32 add per unique value):
  Grev[s] = g_h[E-1-s]          host layout prep: reverse+concat, then per
                                partition p pre-shifted by p//4 zeros
                                (grev_shift[p, s] = Grev[s - p//4])
  S[p, 4s+jr] = grev_shift[p, s] + c_h[p%4, jr]    4x tensor_scalar_add (DVE)
    => S[p, x] = GI_{p%4}[x - 4*(p//4)],  GI_r[4s+jr] = Grev[s] + c[r, jr]
  out[128t+p, j] = S[p, (2044-128t)+j]             ONE 16 MiB DMA

The host-side pre-shift makes the output window start (2044-128t) identical
across partitions, so the bulk store is a single DMA of 2048 contiguous
8 KiB descriptors with a 128-way outer partition dim - it spreads over all
16 SDMA engines and runs at the HBM write roofline (~16 MiB/core).
"""

import numpy as np

_H = 8
_B = 4
_E = 512
_SEQ = 4 * _E              # 2048
_GLEN = 2 * _E - 1         # 1023
_NT = _SEQ // 128          # 16
_SLEN = _GLEN + 31         # 1054: shifted grev row length
_SROW = 4 * _SLEN          # 4216: S row length
_X0 = 4 * (_E - 1)         # 2044: window start for t=0

_CACHE = {}


def _build_nc():
    import concourse.bass as bass
    import concourse.mybir as mybir

    F32 = mybir.dt.float32
    nc = bass.Bass()
    grev_in = nc.dram_tensor("grev", [128, _SLEN], F32, kind="ExternalInput")
    cmat_in = nc.dram_tensor("cmat", [128, 4], F32, kind="ExternalInput")
    out = nc.dram_tensor("out", [_SEQ, _SEQ], F32, kind="ExternalOutput")

    with (
        nc.sbuf_tensor([128, _SLEN], F32) as grev_sb,
        nc.sbuf_tensor([128, 4], F32) as c_sb,
        nc.sbuf_tensor([128, _SROW], F32) as s2,
        nc.semaphore("dma_sem") as dma_sem,
        nc.semaphore("v_sem") as v_sem,
        nc.Block() as block,
    ):
        # S[p, 4s+jr] = grev_shift[p, s] + c[p%4, jr]   (strided dest view)
        s3 = s2[:, :].rearrange("p (s j) -> p s j", j=4)

        @block.vector
        def _(vector):
            vector.wait_ge(dma_sem, 32)  # grev + cmat resident
            for jr in range(4):
                vector.tensor_scalar_add(
                    out=s3[:, :, jr],
                    in0=grev_sb[:, :],
                    scalar1=c_sb[:, jr : jr + 1],
                ).then_inc(v_sem, 1)

        @block.sync
        def _(sync):
            sync.dma_start(out=grev_sb[:, :], in_=grev_in[:, :]).then_inc(dma_sem, 16)
            sync.dma_start(out=c_sb[:, :], in_=cmat_in[:, :]).then_inc(dma_sem, 16)
            sync.wait_ge(v_sem, 4)  # S computed
            # out[128t + p, j] = S[p, (2044 - 128t) + j]
            sb = s2[:, :]
            src = bass.AP(
                sb.tensor,
                sb.offset + _X0,
                [[_SROW, 128], [-128, _NT], [1, _SEQ]],
            )
            dst = out[:, :].rearrange("(t p) j -> p t j", p=128)
            with nc.allow_non_contiguous_dma(reason="toeplitz windows"):
                sync.dma_start(out=dst, in_=src).then_inc(dma_sem, 16)
            sync.wait_ge(dma_sem, 48)

    return nc


def _in_maps(channel_blocks, event_blocks, event_blocks_future):
    maps = []
    for h in range(_H):
        eb = np.ascontiguousarray(event_blocks[:, 0, h], dtype=np.float32)
        ebf = np.ascontiguousarray(event_blocks_future[:, 0, h], dtype=np.float32)
        grev = np.concatenate([eb[_E - 1 : 0 : -1], ebf])  # (1023,)
        # row p: p//4 leading zeros, grev, zeros to length SLEN
        gs = np.zeros((128, _SLEN), dtype=np.float32)
        for q in range(32):
            gs[4 * q : 4 * q + 4, q : q + _GLEN] = grev
        c = np.ascontiguousarray(channel_blocks[:, :, 0, h], dtype=np.float32)  # (4,4)
        maps.append(
            {
                "grev": gs,
                "cmat": np.ascontiguousarray(np.tile(c, (32, 1)), dtype=np.float32),
            }
        )
    return maps


def run_spmd(channel_blocks, event_blocks, event_blocks_future, **spmd_kwargs):
    """Run the per-head kernels on cores 0-7; returns (BassKernelResults, heads).

    heads: float32 (8, 2048, 2048), one bias matrix per head."""
    from concourse.bass_utils import run_bass_kernel_spmd

    if "nc" not in _CACHE:
        _CACHE["nc"] = _build_nc()
    res = run_bass_kernel_spmd(
        _CACHE["nc"],
        _in_maps(channel_blocks, event_blocks, event_blocks_future),
        list(range(_H)),
        **spmd_kwargs,
    )
    heads = np.stack([np.asarray(res.results[h]["out"]) for h in range(_H)])
    return res, heads


def kernel(q, channel_blocks, event_blocks, event_blocks_future):
    q = np.asarray(q)
    channel_blocks = np.asarray(channel_blocks, dtype=np.float32)
    event_blocks = np.asarray(event_blocks, dtype=np.float32)
    event_blocks_future = np.asarray(event_blocks_future, dtype=np.float32)

    _, heads = run_spmd(channel_blocks, event_blocks, event_blocks_future)
    batch = q.shape[0] // _H
    return np.tile(heads, (batch, 1, 1))
